# revision 9
# baseline (speedup 1.0000x reference)
"""Trainium2 Bass kernel for a 2-layer GATv2 (nn_GAT_40372692582770).

Gather-free, PE-centric design (no custom GPSIMD ucode needed):
  - Nodes partitioned by dst range across 8 cores; edges (+self loops)
    routed to the dst owner, sorted by dst, grouped into 64-dst strips,
    padded per strip to a uniform B 128-edge blocks (shared program).
  - Host ships, per layer, per-edge feature columns (the "halo exchange"
    materialized host-side, since the graph is static):
      xsT [65, S]        = x[src_e] columns + valid row   (fp16)
      xdT [65, S]        = x[dst_e] columns + valid row   (fp16)
      xeP [128, NBLK*66] = x[src_e] rows + valid col      (fp16, P-major)
  - Device per chunk (4 strips, CB=4B blocks, CS=CB*128 slots):
      zT[c,e]   = Wl_ext^T xs + Wr_ext^T xd      (PE, feature-major, PSUM)
      L = lrelu(zT, 0.2)  (ACT Lrelu / DVE stt split, fp16)
      e[e,h]    = L_blk^T @ A                    (PE, per 128-edge block)
      w         = exp(e - 2)                     (ACT, fp16)
      oh[e,b,s] = (dstloc == iota64)             (GPSIMD, pads 255 -> 0)
      woh       = oh * w_h  [128, CB, 2, 64]     (DVE)
      gt[j,128] += xe_blk^T @ woh_blk            (PE, per 64-strip, PSUM)
      pair:  sp[128, 130] = sum_h gts_h^T @ R2_h (PE)
             cols = [num_h0 | num_h1 | den_0 den_1]
  - Finalize (interleaved in 4 batches): alpha-normalize, head-mean,
    +bias, gelu -> out_raw fp32 + out_act fp16, both [128, NS*C] P-major.

One program serves both layers (weights are inputs); compiled once.
"""
import os
import sys
import time

sys.path.insert(0, "/opt/trn_rl_repo")

import numpy as np

import concourse.bass as bass
import concourse.mybir as mybir
import concourse.tile as tile
from concourse import bacc
from concourse.bass_utils import run_bass_kernel_spmd

class Cfg:
    N = 100000
    D = 64
    H = 2
    C = 64
    NCORES = 8
    W = 64             # dst nodes per strip
    SPC = 4            # strips per chunk (= 2 pair-strips)
    ESHIFT = -2.0      # exp bias
    LRELU_DVE = ()     # zT group indices whose lrelu runs on DVE (2-pass)

    @property
    def RN(self):
        return self.N // self.NCORES

    @property
    def NSTRIP(self):
        return (self.RN + self.W - 1) // self.W

    @property
    def NPAIR(self):
        return self.NSTRIP // 2

    @property
    def HC(self):
        return self.H * self.C


CFG = Cfg()
FP16 = mybir.dt.float16
FP32 = mybir.dt.float32
AF = mybir.ActivationFunctionType
ALU = mybir.AluOpType


# ------------------------------------------------------------- host prep
def _prep_edges(cfg, src, dst):
    """Route+sort edges, pad strips to B blocks. Returns (B, per-core list
    of (srcids [S], dstloc [S]) in slot order; pad slots src=-1 dstloc=255)."""
    RN = cfg.RN
    core = dst // RN
    shift = int(np.log2(cfg.W))
    per_core = []
    maxblk = 1
    for c in range(cfg.NCORES):
        sel = np.flatnonzero(core == c)
        d = (dst[sel] - c * RN).astype(np.int64)
        s = src[sel].astype(np.int64)
        o = np.argsort(d, kind="stable")
        d, s = d[o], s[o]
        cnt = np.bincount(d >> shift, minlength=cfg.NSTRIP)
        maxblk = max(maxblk, int((cnt.max() + 127) // 128))
        per_core.append((s, d, cnt))
    B = maxblk
    nslot = cfg.NSTRIP * B * 128
    out = []
    for c in range(cfg.NCORES):
        s, d, cnt = per_core[c]
        sids = np.full(nslot, -1, np.int64)
        dloc = np.full(nslot, 255, np.int64)
        pos = 0
        for st in range(cfg.NSTRIP):
            k = int(cnt[st])
            base = st * B * 128
            sids[base:base + k] = s[pos:pos + k]
            dloc[base:base + k] = d[pos:pos + k] & (cfg.W - 1)
            pos += k
        out.append((sids, dloc))
    return B, out


def _prep_layer_weights(cfg, Wl, bl, Wr, br, att, bias):
    D, H, C = cfg.D, cfg.H, cfg.C
    HC = cfg.HC
    # Wsl/Wsr: [D+1, 128] projection for zT (feature-major lhsT)
    wsl = np.zeros((D + 1, HC), np.float64)
    wsl[:D] = Wl
    wsl[D] = bl
    wsr = np.zeros((D + 1, HC), np.float64)
    wsr[:D] = Wr
    wsr[D] = br
    # A [128, 2]: att dot (lrelu applied upstream via ACT Lrelu)
    A = np.zeros((HC, H), np.float64)
    for h in range(H):
        A[h * C:(h + 1) * C, h] = att[h]
    # R2_h [66, 130]: second-level aggregation weights
    R2 = np.zeros((H, 66, 130), np.float64)
    for h in range(H):
        R2[h, :D, h * C:(h + 1) * C] = Wl[:, h * C:(h + 1) * C]
        R2[h, D, h * C:(h + 1) * C] = bl[h * C:(h + 1) * C]
        R2[h, D, HC + h] = 1.0          # denominator column
    biasF = np.tile(bias.astype(np.float32)[None, :], (128, 1))
    return {
        "wsl": wsl.astype(np.float16), "wsr": wsr.astype(np.float16),
        "A": A.astype(np.float16),
        "R2_0": R2[0].astype(np.float16), "R2_1": R2[1].astype(np.float16),
        "biasF": biasF,
    }


# --------------------------------------------------------- program build
def build_program(cfg, B):
    D, H, C = cfg.D, cfg.H, cfg.C
    HC = cfg.HC
    W = cfg.W
    NBLK = cfg.NSTRIP * B
    NCHUNK = cfg.NSTRIP // cfg.SPC
    CB = cfg.SPC * B                   # blocks per chunk
    CS = CB * 128                      # slots per chunk
    S = NBLK * 128
    NS = cfg.NPAIR                     # 128-dst pair strips (stash rows)
    RROW = HC + 2                      # pair psum row [num128 | d0 d1]
    NGRP = CS // 512
    assert CS % 512 == 0

    # finalize batches: after chunk index -> (pair_start, pair_end); a chunk
    # ch produces pairs 2ch, 2ch+1, so cap p1 at 2*(ch+1)
    fin_after = {}
    nb = 4
    p0 = 0
    for i in range(nb):
        ch = (NCHUNK * (i + 1)) // nb - 1
        p1 = 2 * (ch + 1)
        fin_after[ch] = (p0, p1)
        p0 = p1
    assert p0 == NS

    nc = bacc.Bacc("TRN2", target_bir_lowering=False, debug=False,
                   num_devices=cfg.NCORES)

    xsT = nc.declare_dram_parameter("xsT", [D + 1, S], FP16, isOutput=False)
    xdT = nc.declare_dram_parameter("xdT", [D + 1, S], FP16, isOutput=False)
    xeP = nc.declare_dram_parameter("xeP", [128, NBLK * 66], FP16, isOutput=False)
    wsl = nc.declare_dram_parameter("wsl", [D + 1, HC], FP16, isOutput=False)
    wsr = nc.declare_dram_parameter("wsr", [D + 1, HC], FP16, isOutput=False)
    Amat = nc.declare_dram_parameter("A", [HC, H], FP16, isOutput=False)
    R2_0 = nc.declare_dram_parameter("R2_0", [66, 130], FP16, isOutput=False)
    R2_1 = nc.declare_dram_parameter("R2_1", [66, 130], FP16, isOutput=False)
    dstloc = nc.declare_dram_parameter("dstloc", [128, NBLK], FP16, isOutput=False)
    iotaF = nc.declare_dram_parameter("iotaF", [128, W], FP16, isOutput=False)
    biasF = nc.declare_dram_parameter("biasF", [128, C], FP32, isOutput=False)
    out_raw = nc.declare_dram_parameter("out_raw", [128, NS * C], FP32,
                                        isOutput=True)
    out_act = nc.declare_dram_parameter("out_act", [128, NS * C], FP16,
                                        isOutput=True)

    with tile.TileContext(nc) as tc:
        with (
            tc.tile_pool(name="const", bufs=1) as cpool,
            tc.tile_pool(name="stash", bufs=1) as stpool,
            tc.tile_pool(name="fin", bufs=1) as fpool,
        ):
            wsl_t = cpool.tile([D + 1, HC], FP16)
            nc.sync.dma_start(out=wsl_t[:], in_=wsl[:, :])
            wsr_t = cpool.tile([D + 1, HC], FP16)
            nc.sync.dma_start(out=wsr_t[:], in_=wsr[:, :])
            A_t = cpool.tile([HC, H], FP16)
            nc.sync.dma_start(out=A_t[:], in_=Amat[:, :])
            r2_t = [cpool.tile([66, 130], FP16, tag=f"r2{h}", name=f"r2{h}")
                    for h in range(H)]
            nc.sync.dma_start(out=r2_t[0][:], in_=R2_0[:, :])
            nc.sync.dma_start(out=r2_t[1][:], in_=R2_1[:, :])
            dl_t = cpool.tile([128, NBLK], FP16)
            nc.sync.dma_start(out=dl_t[:], in_=dstloc[:, :])
            iota_t = cpool.tile([128, W], FP16)
            nc.sync.dma_start(out=iota_t[:], in_=iotaF[:, :])
            ebias_t = cpool.tile([128, 1], FP32)
            nc.vector.memset(ebias_t[:], cfg.ESHIFT)
            bias_t = cpool.tile([128, C], FP32)
            nc.sync.dma_start(out=bias_t[:], in_=biasF[:, :])

            stash = stpool.tile([128, NS * RROW], FP32)
            sv = stash[:].rearrange("p (s w) -> p s w", w=RROW)

            # finalize working tiles (batch-sliced)
            rec = fpool.tile([128, NS * 2], FP32, tag="rec")
            recv = rec[:].rearrange("p (s k) -> p s k", k=2)
            tmean = fpool.tile([128, NS * C], FP32, tag="tmean")
            tm = tmean[:].rearrange("p (s c) -> p s c", c=C)
            t2 = fpool.tile([128, NS * C], FP32, tag="t2")
            t2v = t2[:].rearrange("p (s c) -> p s c", c=C)
            cub = fpool.tile([128, NS * C], FP32, tag="cub")
            cv = cub[:].rearrange("p (s c) -> p s c", c=C)
            outg = fpool.tile([128, NS * C], FP16, tag="outg")
            ogv = outg[:].rearrange("p (s c) -> p s c", c=C)

            with (
                tc.tile_pool(name="eg", bufs=2) as egpool,
                tc.tile_pool(name="ez", bufs=2) as ezpool,
                tc.tile_pool(name="esm", bufs=3) as smpool,
                tc.tile_pool(name="zps", bufs=2, space="PSUM") as zpspool,
                tc.tile_pool(name="eps", bufs=2, space="PSUM") as epspool,
                tc.tile_pool(name="gps", bufs=2, space="PSUM") as gpspool,
                tc.tile_pool(name="sps", bufs=2, space="PSUM") as spspool,
            ):
                for ch in range(NCHUNK):
                    c0 = ch * CS
                    b0 = ch * CB
                    xs_t = egpool.tile([D + 1, CS], FP16, tag="xs")
                    nc.sync.dma_start(out=xs_t[:], in_=xsT[:, c0:c0 + CS])
                    xd_t = egpool.tile([D + 1, CS], FP16, tag="xd")
                    nc.sync.dma_start(out=xd_t[:], in_=xdT[:, c0:c0 + CS])
                    xe_t = egpool.tile([128, CB * 66], FP16, tag="xe")
                    nc.sync.dma_start(out=xe_t[:],
                                      in_=xeP[:, b0 * 66:(b0 + CB) * 66])
                    xev = xe_t[:].rearrange("p (b w) -> p b w", w=66)

                    # zT feature-major in groups of 512 edges; L = lrelu(zT)
                    L = ezpool.tile([128, CS], FP16, tag="L")
                    for g in range(NGRP):
                        g0 = g * 512
                        zp = zpspool.tile([128, 512], FP32, tag="zp")
                        nc.tensor.matmul(zp[:], lhsT=wsl_t[:],
                                         rhs=xs_t[:, g0:g0 + 512],
                                         start=True, stop=False)
                        nc.tensor.matmul(zp[:], lhsT=wsr_t[:],
                                         rhs=xd_t[:, g0:g0 + 512],
                                         start=False, stop=True)
                        if g in cfg.LRELU_DVE:
                            ztmp = smpool.tile([128, 512], FP32, tag="ztmp")
                            nc.vector.tensor_scalar_mul(ztmp[:], zp[:], 0.2)
                            nc.vector.tensor_tensor(
                                out=L[:, g0:g0 + 512], in0=ztmp[:],
                                in1=zp[:], op=ALU.max)
                        else:
                            nc.scalar.activation(out=L[:, g0:g0 + 512],
                                                 in_=zp[:], func=AF.Prelu,
                                                 alpha=0.2)

                    # e-dot per block -> e psum [128, 2*CB]
                    ep = epspool.tile([128, 2 * CB], FP32, tag="ep")
                    for b in range(CB):
                        nc.tensor.matmul(ep[:, 2 * b:2 * b + 2],
                                         lhsT=L[:, b * 128:(b + 1) * 128],
                                         rhs=A_t[:], start=True, stop=True)
                    w_t = smpool.tile([128, 2 * CB], FP16, tag="w")
                    wv = w_t[:].rearrange("p (b k) -> p b k", k=2)
                    nc.scalar.activation(out=w_t[:], in_=ep[:], func=AF.Exp,
                                         bias=ebias_t[:])

                    # onehot (DVE) + woh [128, CB, 2, 64] (Pool/DVE split —
                    # Pool TensorTensor only supports mult/subtract)
                    oh = ezpool.tile([128, CB * W], FP16, tag="oh")
                    ohv = oh[:].rearrange("p (b s) -> p b s", s=W)
                    nc.vector.tensor_tensor(
                        out=ohv[:, :, :],
                        in0=dl_t[:, b0:b0 + CB].unsqueeze(2).to_broadcast(
                            [128, CB, W]),
                        in1=iota_t[:].unsqueeze(1).to_broadcast([128, CB, W]),
                        op=ALU.is_equal)
                    woh = ezpool.tile([128, CB * 2 * W], FP16, tag="woh")
                    wohv = woh[:].rearrange("p (b h s) -> p b h s", h=2, s=W)
                    KB = (2 * CB) // 3
                    for eng, lo, hi in ((nc.gpsimd, 0, KB),
                                        (nc.vector, KB, CB)):
                        eng.tensor_tensor(
                            out=wohv[:, lo:hi, :, :],
                            in0=ohv[:, lo:hi, :].unsqueeze(2).to_broadcast(
                                [128, hi - lo, 2, W]),
                            in1=wv[:, lo:hi, :].unsqueeze(3).to_broadcast(
                                [128, hi - lo, 2, W]),
                            op=ALU.mult)

                    # GT per 64-strip; pairs -> sp -> stash
                    for s4 in range(cfg.SPC):
                        half = s4 % 2
                        if half == 0:
                            gts = smpool.tile([66, 256], FP16, tag="gts")
                        gt = gpspool.tile([66, 128], FP32, tag="gt")
                        for b in range(B):
                            blk = s4 * B + b
                            nc.tensor.matmul(
                                gt[:], lhsT=xev[:, blk, :],
                                rhs=wohv[:, blk, :, :],
                                start=(b == 0), stop=(b == B - 1))
                        # gts h-major: cols = h*128 + half*64 + w, so each
                        # head's 128 cols are contiguous (1-free-dim lhsT)
                        eng = nc.vector if (s4 % 2 == 0) else nc.scalar
                        gtv = gt[:].rearrange("p (h w) -> p h w", h=2, w=W)
                        gtsv = gts[:].rearrange(
                            "p (h s w) -> p h s w", h=2, s=2, w=W)
                        dst_sl = gtsv[:, :, half, :]
                        if eng is nc.vector:
                            nc.vector.tensor_copy(dst_sl, gtv[:, :, :])
                        else:
                            nc.scalar.copy(dst_sl, gtv[:, :, :])
                        if half == 1:
                            pr = ch * 2 + s4 // 2
                            sp = spspool.tile([128, RROW], FP32, tag="sp")
                            nc.tensor.matmul(sp[:], lhsT=gts[:, 0:128],
                                             rhs=r2_t[0][:],
                                             start=True, stop=False)
                            nc.tensor.matmul(sp[:], lhsT=gts[:, 128:256],
                                             rhs=r2_t[1][:],
                                             start=False, stop=True)
                            dst_sl = stash[:, pr * RROW:(pr + 1) * RROW]
                            eng2 = nc.scalar if (s4 // 2 == 0) else nc.vector
                            if eng2 is nc.vector:
                                nc.vector.tensor_copy(dst_sl, sp[:])
                            else:
                                nc.scalar.copy(dst_sl, sp[:])

                    # ---------------- finalize batch ----------------
                    if ch in fin_after:
                        p0, p1 = fin_after[ch]
                        NSb = p1 - p0
                        sl = slice(p0, p1)
                        nc.vector.reciprocal(out=recv[:, sl, :],
                                             in_=sv[:, sl, HC:HC + 2])
                        nc.vector.tensor_tensor(
                            out=tm[:, sl, :], in0=sv[:, sl, 0:C],
                            in1=recv[:, sl, 0:1].to_broadcast([128, NSb, C]),
                            op=ALU.mult)
                        nc.vector.tensor_tensor(
                            out=t2v[:, sl, :], in0=sv[:, sl, C:2 * C],
                            in1=recv[:, sl, 1:2].to_broadcast([128, NSb, C]),
                            op=ALU.mult)
                        nc.vector.tensor_tensor(out=tm[:, sl, :],
                                                in0=tm[:, sl, :],
                                                in1=t2v[:, sl, :], op=ALU.add)
                        # tm = 0.5*tm + bias
                        nc.vector.scalar_tensor_tensor(
                            out=tm[:, sl, :], in0=tm[:, sl, :], scalar=0.5,
                            in1=bias_t[:].unsqueeze(1).to_broadcast(
                                [128, NSb, C]),
                            op0=ALU.mult, op1=ALU.add)
                        # gelu_tanh(x) = x*sigmoid(2*sqrt(2/pi)*(x+0.044715x^3))
                        nc.scalar.square(cv[:, sl, :], tm[:, sl, :])
                        nc.vector.tensor_tensor(out=cv[:, sl, :],
                                                in0=cv[:, sl, :],
                                                in1=tm[:, sl, :], op=ALU.mult)
                        nc.vector.scalar_tensor_tensor(
                            out=cv[:, sl, :], in0=cv[:, sl, :],
                            scalar=0.044715, in1=tm[:, sl, :],
                            op0=ALU.mult, op1=ALU.add)
                        nc.scalar.activation(out=cv[:, sl, :],
                                             in_=cv[:, sl, :],
                                             func=AF.Sigmoid,
                                             scale=1.5957691216057308)
                        nc.vector.tensor_tensor(out=ogv[:, sl, :],
                                                in0=cv[:, sl, :],
                                                in1=tm[:, sl, :], op=ALU.mult)
                        nc.sync.dma_start(
                            out=out_raw[:, p0 * C:p1 * C],
                            in_=tmean[:, p0 * C:p1 * C])
                        nc.sync.dma_start(
                            out=out_act[:, p0 * C:p1 * C],
                            in_=outg[:, p0 * C:p1 * C])

    nc.compile()
    return nc




# ----------------------------------------------------- persistent runner
class Runner:
    """Jit-compiled SPMD callable with reusable device inputs (no donation)."""

    def __init__(self, nc, n_cores):
        import jax
        import concourse.mybir as mb
        from concourse import bass2jax
        from jax.experimental.shard_map import shard_map
        from jax.sharding import Mesh, PartitionSpec
        bass2jax.install_neuronx_cc_hook()
        self.nc = nc
        self.n_cores = n_cores
        in_names, out_names, out_avals, zero_outs = [], [], [], []
        for alloc in nc.m.functions[0].allocations:
            if not isinstance(alloc, mb.MemoryLocationSet):
                continue
            name = alloc.memorylocations[0].name
            if alloc.kind == "ExternalInput":
                in_names.append(name)
            elif alloc.kind == "ExternalOutput":
                out_names.append(name)
                shape = tuple(alloc.tensor_shape)
                dtype = mb.dt.np(alloc.dtype)
                out_avals.append(jax.core.ShapedArray(shape, dtype))
                zero_outs.append(np.zeros(shape, dtype))
        pt = nc.partition_id_tensor
        self.pname = pt.name if pt else None
        if self.pname in in_names:
            in_names.remove(self.pname)
        self.in_names = list(in_names)
        self.out_names = list(out_names)
        self.out_avals = out_avals
        self.zero_outs = zero_outs
        all_in = list(in_names) + list(out_names)
        if self.pname:
            all_in.append(self.pname)

        def _body(*args):
            operands = list(args)
            if self.pname:
                operands.append(bass2jax.partition_id_tensor())
            outs = bass2jax._bass_exec_p.bind(
                *operands,
                out_avals=tuple(out_avals),
                in_names=tuple(all_in),
                out_names=tuple(out_names),
                lowering_input_output_aliases=(),
                sim_require_finite=True,
                sim_require_nnan=True,
                nc=nc,
            )
            return tuple(outs)

        devices = jax.devices()[:n_cores]
        self.mesh = Mesh(np.asarray(devices), ("core",))
        np_in = (PartitionSpec("core"),) * (len(in_names) + len(out_names))
        np_out = (PartitionSpec("core"),) * len(out_names)
        self.fn = jax.jit(shard_map(_body, mesh=self.mesh, in_specs=np_in,
                                    out_specs=np_out, check_rep=False),
                          keep_unused=True)

    def put(self, in_maps):
        """Concat per-core inputs and move to device. Returns arg list."""
        import jax
        from jax.sharding import NamedSharding, PartitionSpec
        sh = NamedSharding(self.mesh, PartitionSpec("core"))
        args = []
        for name in self.in_names:
            cat = np.concatenate([np.asarray(m[name]) for m in in_maps], axis=0)
            args.append(jax.device_put(cat, sh))
        for z in self.zero_outs:
            zz = np.zeros((self.n_cores * z.shape[0], *z.shape[1:]), z.dtype)
            args.append(jax.device_put(zz, sh))
        return args

    def run(self, args):
        return self.fn(*args)

    def results(self, out_arrs):
        res = []
        for c in range(self.n_cores):
            res.append({
                name: np.asarray(out_arrs[i]).reshape(
                    self.n_cores, *self.out_avals[i].shape)[c]
                for i, name in enumerate(self.out_names)})
        return res


# ------------------------------------------------------------- kernel()
_CACHE = {}
_RUNNERS = {}
LAST_ARGS = None
LAST_LAUNCH_NS = None


def build_in_map(cfg, cur, slots, lw, iotaF, dlocP, c):
    B = len(slots[c][0]) // (cfg.NSTRIP * 128)
    S = cfg.NSTRIP * B * 128
    NBLK = cfg.NSTRIP * B
    sids, dloc = slots[c]
    pad = sids < 0
    xs = cur[np.where(pad, 0, sids)]
    xs[pad] = 0
    xsT = np.empty((cfg.D + 1, S), np.float16)
    xsT[:cfg.D] = xs.T
    xsT[cfg.D] = (~pad).astype(np.float16)
    # xeP [128, NBLK*66]: partition-major rows of xs
    xeP = np.zeros((NBLK, 128, 66), np.float16)
    xeP[:, :, :cfg.D] = xs.reshape(NBLK, 128, cfg.D)
    xeP[:, :, cfg.D] = (~pad).astype(np.float16).reshape(NBLK, 128)
    xeP = np.ascontiguousarray(
        xeP.transpose(1, 0, 2).reshape(128, NBLK * 66))
    # dst features: dst node id = strip*W + (dloc & W-1); recompute global
    strip = np.arange(S) // (B * 128)
    dlg = np.where(pad, 0, strip * cfg.W + dloc)
    xd = cur[c * cfg.RN + np.minimum(dlg, cfg.RN - 1)]
    xd[pad] = 0
    xdT = np.empty((cfg.D + 1, S), np.float16)
    xdT[:cfg.D] = xd.T
    xdT[cfg.D] = (~pad).astype(np.float16)
    return {
        "xsT": xsT, "xdT": xdT, "xeP": xeP,
        "wsl": lw["wsl"], "wsr": lw["wsr"], "A": lw["A"],
        "R2_0": lw["R2_0"], "R2_1": lw["R2_1"],
        "dstloc": dlocP[c], "iotaF": iotaF, "biasF": lw["biasF"],
    }


def prep_all(cfg, src, dst):
    B, slots = _prep_edges(cfg, src, dst)
    dlocP = []
    for c in range(cfg.NCORES):
        sids, dloc = slots[c]
        NBLK = cfg.NSTRIP * B
        dlocP.append(np.ascontiguousarray(
            dloc.reshape(NBLK, 128).T.astype(np.float16)))
    return B, slots, dlocP


def kernel(embeded_nodes_features, edges_connectivity, Wl, bl, Wr, br, att, bias):
    global LAST_LAUNCH_NS
    cfg = CFG
    x = np.asarray(embeded_nodes_features, np.float32)
    ec = np.asarray(edges_connectivity)
    src = np.concatenate([ec[0], np.arange(cfg.N, dtype=ec.dtype)]).astype(np.int64)
    dst = np.concatenate([ec[1], np.arange(cfg.N, dtype=ec.dtype)]).astype(np.int64)
    Wl = np.asarray(Wl, np.float32)
    bl = np.asarray(bl, np.float32)
    Wr = np.asarray(Wr, np.float32)
    br = np.asarray(br, np.float32)
    att = np.asarray(att, np.float32)
    bias = np.asarray(bias, np.float32)
    L = Wl.shape[0]

    B, slots, dlocP = prep_all(cfg, src, dst)
    if B not in _CACHE:
        _CACHE[B] = build_program(cfg, B)
    prog = _CACHE[B]
    lws = [_prep_layer_weights(cfg, Wl[i], bl[i], Wr[i], br[i], att[i], bias[i])
           for i in range(L)]
    iotaF = np.tile(np.arange(cfg.W, dtype=np.float16)[None, :], (128, 1))

    cur = x.astype(np.float16)
    out_full = None
    _launch_ns = []
    _args_hist = []
    NS = cfg.NPAIR
    for i in range(L):
        lw = lws[i]
        in_maps = [build_in_map(cfg, cur, slots, lw, iotaF, dlocP, c)
                   for c in range(cfg.NCORES)]
        if B not in _RUNNERS:
            _RUNNERS[B] = Runner(prog, cfg.NCORES)
        runner = _RUNNERS[B]
        args = runner.put(in_maps)
        _args_hist.append(args)
        t0 = time.time()
        outs = runner.run(args)
        import jax
        jax.block_until_ready(outs)
        _launch_ns.append(int((time.time() - t0) * 1e9))
        res = runner.results(outs)

        def unpack(a):
            return a.reshape(128, NS, cfg.C).swapaxes(0, 1).reshape(
                NS * 128, cfg.C)[:cfg.RN]
        raw = np.concatenate(
            [unpack(res[c]["out_raw"]) for c in range(cfg.NCORES)], axis=0)
        actv = np.concatenate(
            [unpack(res[c]["out_act"]) for c in range(cfg.NCORES)], axis=0)
        out_full = raw
        cur = actv
    LAST_LAUNCH_NS = _launch_ns
    global LAST_ARGS
    LAST_ARGS = _args_hist
    return out_full.astype(np.float32)


# revision 10
# speedup vs baseline: 5.8109x; 5.8109x over previous
"""Trainium2 Bass kernel for a 2-layer GATv2 (nn_GAT_40372692582770).

Gather-free, PE-centric design:
  - Nodes partitioned by dst range across 8 cores; edges (+self loops)
    routed to the dst owner, sorted by dst, grouped into 64-dst strips.
  - Strips are sorted per core by edge count (host permutation); the SPMD
    block schedule sched[i] = max over cores of ceil(cnt_sorted/128) is
    shared, giving ~10% slot padding instead of 33% at uniform B.
  - Host ships, per layer, per-edge feature columns (the "halo exchange"
    materialized host-side, since the graph is static):
      xsT [65, S]        = x[src_e] columns + valid row   (fp16)
      xdT [65, S]        = x[dst_e] columns + valid row   (fp16)
      xeP [128, NBLK*66] = x[src_e] rows + valid col      (fp16, P-major)
  - Device per pair of strips (bp = bA+bB blocks, ~1100 slots):
      zT[c,e]   = Wl_ext^T xs + Wr_ext^T xd      (PE, feature-major, PSUM)
      L = lrelu(zT, 0.2)  (ACT Prelu, fp16)
      e[e,h]    = L_blk^T @ A                    (PE, per 128-edge block)
      w         = exp(e - 2)                     (ACT, fp16)
      oh[e,b,s] = (dstloc == iota64)             (DVE, pads 255 -> 0)
      woh       = oh * w_h  [128, b, 2, 64]      (Pool/DVE split)
      gt[j,128] += xe_blk^T @ woh_blk            (PE, per strip, PSUM)
      pair:  sp[128, 130] = sum_h gts_h^T @ R2_h (PE)
             cols = [num_h0 | num_h1 | den_0 den_1]
  - Finalize (interleaved in 8 batches): alpha-normalize, head-mean,
    +bias, gelu -> out_raw fp32 + out_act fp16, both [128, NS*C] P-major
    in sorted-strip order (host unpermutes).

One program serves both layers (weights are inputs); compiled once per
block schedule.
"""
import os
import sys
import time

sys.path.insert(0, "/opt/trn_rl_repo")

import numpy as np

import concourse.bass as bass
import concourse.mybir as mybir
import concourse.tile as tile
from concourse import bacc
from concourse.bass_utils import run_bass_kernel_spmd

class Cfg:
    N = 100000
    D = 64
    H = 2
    C = 64
    NCORES = 8
    W = 64             # dst nodes per strip
    ESHIFT = -2.0      # exp bias
    NFIN = 8           # finalize batches

    @property
    def RN(self):
        return self.N // self.NCORES

    @property
    def NSTRIP(self):
        return (self.RN + self.W - 1) // self.W

    @property
    def NPAIR(self):
        return self.NSTRIP // 2

    @property
    def HC(self):
        return self.H * self.C


CFG = Cfg()
FP16 = mybir.dt.float16
FP32 = mybir.dt.float32
AF = mybir.ActivationFunctionType
ALU = mybir.AluOpType


# ------------------------------------------------------------- host prep
def _prep_edges(cfg, src, dst):
    """Route+sort edges; strips sorted by count per core; shared schedule.

    Returns (sched [NSTRIP], perms: per-core strip permutation,
    slots: per-core (srcids [S], dstloc [S]); pad slots src=-1 dstloc=255).
    """
    RN = cfg.RN
    core = dst // RN
    shift = int(np.log2(cfg.W))
    per_core = []
    cnts = np.zeros((cfg.NCORES, cfg.NSTRIP), np.int64)
    for c in range(cfg.NCORES):
        sel = np.flatnonzero(core == c)
        d = (dst[sel] - c * RN).astype(np.int64)
        s = src[sel].astype(np.int64)
        o = np.argsort(d, kind="stable")
        d, s = d[o], s[o]
        cnts[c] = np.bincount(d >> shift, minlength=cfg.NSTRIP)
        per_core.append((s, d))
    perms = [np.argsort(-cnts[c], kind="stable") for c in range(cfg.NCORES)]
    sorted_cnt = np.stack([cnts[c][perms[c]] for c in range(cfg.NCORES)])
    sched = np.maximum(1, (sorted_cnt.max(axis=0) + 127) // 128).astype(int)
    boff = np.concatenate([[0], np.cumsum(sched)])
    nslot = int(boff[-1]) * 128
    # strip start offsets (in edge-sorted order) per core
    out = []
    for c in range(cfg.NCORES):
        s, d = per_core[c]
        starts = np.concatenate([[0], np.cumsum(cnts[c])])
        sids = np.full(nslot, -1, np.int64)
        dloc = np.full(nslot, 255, np.int64)
        for i in range(cfg.NSTRIP):
            st = perms[c][i]
            k = int(cnts[c][st])
            base = int(boff[i]) * 128
            e0 = int(starts[st])
            sids[base:base + k] = s[e0:e0 + k]
            dloc[base:base + k] = d[e0:e0 + k] & (cfg.W - 1)
        out.append((sids, dloc))
    return sched, perms, out


def _prep_layer_weights(cfg, Wl, bl, Wr, br, att, bias):
    D, H, C = cfg.D, cfg.H, cfg.C
    HC = cfg.HC
    wsl = np.zeros((D + 1, HC), np.float64)
    wsl[:D] = Wl
    wsl[D] = bl
    wsr = np.zeros((D + 1, HC), np.float64)
    wsr[:D] = Wr
    wsr[D] = br
    A = np.zeros((HC, H), np.float64)
    for h in range(H):
        A[h * C:(h + 1) * C, h] = att[h]
    R2 = np.zeros((H, 66, 130), np.float64)
    for h in range(H):
        R2[h, :D, h * C:(h + 1) * C] = Wl[:, h * C:(h + 1) * C]
        R2[h, D, h * C:(h + 1) * C] = bl[h * C:(h + 1) * C]
        R2[h, D, HC + h] = 1.0          # denominator column
    biasF = np.tile(bias.astype(np.float32)[None, :], (128, 1))
    return {
        "wsl": wsl.astype(np.float16), "wsr": wsr.astype(np.float16),
        "A": A.astype(np.float16),
        "R2_0": R2[0].astype(np.float16), "R2_1": R2[1].astype(np.float16),
        "biasF": biasF,
    }


# --------------------------------------------------------- program build
def build_program(cfg, sched):
    D, H, C = cfg.D, cfg.H, cfg.C
    HC = cfg.HC
    W = cfg.W
    sched = [int(b) for b in sched]
    boff = [0]
    for b in sched:
        boff.append(boff[-1] + b)
    NBLK = boff[-1]
    S = NBLK * 128
    NS = cfg.NPAIR
    RROW = HC + 2
    BPMAX = max(sched[2 * p] + sched[2 * p + 1] for p in range(NS))
    CSMAX = BPMAX * 128

    # finalize batches: after pair index -> (pair_start, pair_end)
    fin_after = {}
    p0 = 0
    for i in range(cfg.NFIN):
        pe_ = (NS * (i + 1)) // cfg.NFIN
        fin_after[pe_ - 1] = (p0, pe_)
        p0 = pe_

    nc = bacc.Bacc("TRN2", target_bir_lowering=False, debug=False,
                   num_devices=cfg.NCORES)

    xsT = nc.declare_dram_parameter("xsT", [D + 1, S], FP16, isOutput=False)
    xdT = nc.declare_dram_parameter("xdT", [D + 1, S], FP16, isOutput=False)
    xeP = nc.declare_dram_parameter("xeP", [128, NBLK * 66], FP16, isOutput=False)
    wsl = nc.declare_dram_parameter("wsl", [D + 1, HC], FP16, isOutput=False)
    wsr = nc.declare_dram_parameter("wsr", [D + 1, HC], FP16, isOutput=False)
    Amat = nc.declare_dram_parameter("A", [HC, H], FP16, isOutput=False)
    R2_0 = nc.declare_dram_parameter("R2_0", [66, 130], FP16, isOutput=False)
    R2_1 = nc.declare_dram_parameter("R2_1", [66, 130], FP16, isOutput=False)
    dstloc = nc.declare_dram_parameter("dstloc", [128, NBLK], FP16, isOutput=False)
    iotaF = nc.declare_dram_parameter("iotaF", [128, W], FP16, isOutput=False)
    biasF = nc.declare_dram_parameter("biasF", [128, C], FP32, isOutput=False)
    out_raw = nc.declare_dram_parameter("out_raw", [128, NS * C], FP32,
                                        isOutput=True)
    out_act = nc.declare_dram_parameter("out_act", [128, NS * C], FP16,
                                        isOutput=True)

    with tile.TileContext(nc) as tc:
        with (
            tc.tile_pool(name="const", bufs=1) as cpool,
            tc.tile_pool(name="stash", bufs=1) as stpool,
            tc.tile_pool(name="fin", bufs=1) as fpool,
        ):
            wsl_t = cpool.tile([D + 1, HC], FP16)
            nc.sync.dma_start(out=wsl_t[:], in_=wsl[:, :])
            wsr_t = cpool.tile([D + 1, HC], FP16)
            nc.sync.dma_start(out=wsr_t[:], in_=wsr[:, :])
            A_t = cpool.tile([HC, H], FP16)
            nc.sync.dma_start(out=A_t[:], in_=Amat[:, :])
            r2_t = [cpool.tile([66, 130], FP16, tag=f"r2{h}", name=f"r2{h}")
                    for h in range(H)]
            nc.sync.dma_start(out=r2_t[0][:], in_=R2_0[:, :])
            nc.sync.dma_start(out=r2_t[1][:], in_=R2_1[:, :])
            dl_t = cpool.tile([128, NBLK], FP16)
            nc.sync.dma_start(out=dl_t[:], in_=dstloc[:, :])
            iota_t = cpool.tile([128, W], FP16)
            nc.sync.dma_start(out=iota_t[:], in_=iotaF[:, :])
            ebias_t = cpool.tile([128, 1], FP32)
            nc.vector.memset(ebias_t[:], cfg.ESHIFT)
            bias_t = cpool.tile([128, C], FP32)
            nc.sync.dma_start(out=bias_t[:], in_=biasF[:, :])

            stash = stpool.tile([128, NS * RROW], FP32)
            sv = stash[:].rearrange("p (s w) -> p s w", w=RROW)

            rec = fpool.tile([128, NS * 2], FP32, tag="rec")
            recv = rec[:].rearrange("p (s k) -> p s k", k=2)
            tmean = fpool.tile([128, NS * C], FP32, tag="tmean")
            tm = tmean[:].rearrange("p (s c) -> p s c", c=C)
            t2 = fpool.tile([128, NS * C], FP32, tag="t2")
            t2v = t2[:].rearrange("p (s c) -> p s c", c=C)
            cub = fpool.tile([128, NS * C], FP32, tag="cub")
            cv = cub[:].rearrange("p (s c) -> p s c", c=C)
            outg = fpool.tile([128, NS * C], FP16, tag="outg")
            ogv = outg[:].rearrange("p (s c) -> p s c", c=C)

            with (
                tc.tile_pool(name="eg", bufs=3) as egpool,
                tc.tile_pool(name="ez", bufs=3) as ezpool,
                tc.tile_pool(name="esm", bufs=3) as smpool,
                tc.tile_pool(name="zps", bufs=2, space="PSUM") as zpspool,
                tc.tile_pool(name="eps", bufs=2, space="PSUM") as epspool,
                tc.tile_pool(name="gps", bufs=2, space="PSUM") as gpspool,
                tc.tile_pool(name="sps", bufs=2, space="PSUM") as spspool,
            ):
                for pr in range(NS):
                    bA = sched[2 * pr]
                    bB = sched[2 * pr + 1]
                    bp = bA + bB
                    b0 = boff[2 * pr]
                    c0 = b0 * 128
                    CS = bp * 128
                    xs_t = egpool.tile([D + 1, CSMAX], FP16, tag="xs")
                    nc.sync.dma_start(out=xs_t[:, :CS],
                                      in_=xsT[:, c0:c0 + CS])
                    xd_t = egpool.tile([D + 1, CSMAX], FP16, tag="xd")
                    nc.sync.dma_start(out=xd_t[:, :CS],
                                      in_=xdT[:, c0:c0 + CS])
                    xe_t = egpool.tile([128, BPMAX * 66], FP16, tag="xe")
                    nc.sync.dma_start(out=xe_t[:, :bp * 66],
                                      in_=xeP[:, b0 * 66:(b0 + bp) * 66])
                    xev = xe_t[:, :bp * 66].rearrange("p (b w) -> p b w", w=66)

                    # zT feature-major in groups of <=512; L = lrelu(zT)
                    L = ezpool.tile([128, CSMAX], FP16, tag="L")
                    ngrp = (CS + 511) // 512
                    for g in range(ngrp):
                        g0 = g * 512
                        gw = min(512, CS - g0)
                        zp = zpspool.tile([128, 512], FP32, tag="zp")
                        nc.tensor.matmul(zp[:, :gw], lhsT=wsl_t[:],
                                         rhs=xs_t[:, g0:g0 + gw],
                                         start=True, stop=False)
                        nc.tensor.matmul(zp[:, :gw], lhsT=wsr_t[:],
                                         rhs=xd_t[:, g0:g0 + gw],
                                         start=False, stop=True)
                        nc.scalar.activation(out=L[:, g0:g0 + gw],
                                             in_=zp[:, :gw], func=AF.Prelu,
                                             alpha=0.2)

                    # e-dot per block -> e psum [128, 2*bp]
                    ep = epspool.tile([128, 2 * BPMAX], FP32, tag="ep")
                    for b in range(bp):
                        nc.tensor.matmul(ep[:, 2 * b:2 * b + 2],
                                         lhsT=L[:, b * 128:(b + 1) * 128],
                                         rhs=A_t[:], start=True, stop=True)
                    w_t = smpool.tile([128, 2 * BPMAX], FP16, tag="w")
                    wv = w_t[:].rearrange("p (b k) -> p b k", k=2)
                    nc.scalar.activation(out=w_t[:, :2 * bp],
                                         in_=ep[:, :2 * bp], func=AF.Exp,
                                         bias=ebias_t[:])

                    # onehot (DVE) + woh [128, b, 2, 64] (Pool/DVE split)
                    oh = ezpool.tile([128, BPMAX * W], FP16, tag="oh")
                    ohv = oh[:].rearrange("p (b s) -> p b s", s=W)
                    nc.vector.tensor_tensor(
                        out=ohv[:, :bp, :],
                        in0=dl_t[:, b0:b0 + bp].unsqueeze(2).to_broadcast(
                            [128, bp, W]),
                        in1=iota_t[:].unsqueeze(1).to_broadcast([128, bp, W]),
                        op=ALU.is_equal)
                    woh = ezpool.tile([128, BPMAX * 2 * W], FP16, tag="woh")
                    wohv = woh[:].rearrange("p (b h s) -> p b h s", h=2, s=W)
                    KB = (2 * bp) // 3
                    for eng, lo, hi in ((nc.gpsimd, 0, KB),
                                        (nc.vector, KB, bp)):
                        if hi <= lo:
                            continue
                        eng.tensor_tensor(
                            out=wohv[:, lo:hi, :, :],
                            in0=ohv[:, lo:hi, :].unsqueeze(2).to_broadcast(
                                [128, hi - lo, 2, W]),
                            in1=wv[:, lo:hi, :].unsqueeze(3).to_broadcast(
                                [128, hi - lo, 2, W]),
                            op=ALU.mult)

                    # GT per strip; pair -> sp -> stash
                    gts = smpool.tile([66, 256], FP16, tag="gts")
                    gtsv = gts[:].rearrange("p (h s w) -> p h s w",
                                            h=2, s=2, w=W)
                    for half, nb in ((0, bA), (1, bB)):
                        gt = gpspool.tile([66, 128], FP32, tag="gt")
                        base = 0 if half == 0 else bA
                        for b in range(nb):
                            blk = base + b
                            nc.tensor.matmul(
                                gt[:], lhsT=xev[:, blk, :],
                                rhs=wohv[:, blk, :, :],
                                start=(b == 0), stop=(b == nb - 1))
                        gtv = gt[:].rearrange("p (h w) -> p h w", h=2, w=W)
                        dst_sl = gtsv[:, :, half, :]
                        if half == 0:
                            nc.vector.tensor_copy(dst_sl, gtv[:, :, :])
                        else:
                            nc.scalar.copy(dst_sl, gtv[:, :, :])
                    sp = spspool.tile([128, RROW], FP32, tag="sp")
                    nc.tensor.matmul(sp[:], lhsT=gts[:, 0:128],
                                     rhs=r2_t[0][:], start=True, stop=False)
                    nc.tensor.matmul(sp[:], lhsT=gts[:, 128:256],
                                     rhs=r2_t[1][:], start=False, stop=True)
                    dst_sl = stash[:, pr * RROW:(pr + 1) * RROW]
                    if pr % 2 == 0:
                        nc.scalar.copy(dst_sl, sp[:])
                    else:
                        nc.vector.tensor_copy(dst_sl, sp[:])

                    # ---------------- finalize batch ----------------
                    if pr in fin_after:
                        p0, p1 = fin_after[pr]
                        NSb = p1 - p0
                        sl = slice(p0, p1)
                        nc.vector.reciprocal(out=recv[:, sl, :],
                                             in_=sv[:, sl, HC:HC + 2])
                        nc.vector.tensor_tensor(
                            out=tm[:, sl, :], in0=sv[:, sl, 0:C],
                            in1=recv[:, sl, 0:1].to_broadcast([128, NSb, C]),
                            op=ALU.mult)
                        nc.gpsimd.tensor_tensor(
                            out=t2v[:, sl, :], in0=sv[:, sl, C:2 * C],
                            in1=recv[:, sl, 1:2].to_broadcast([128, NSb, C]),
                            op=ALU.mult)
                        nc.vector.tensor_tensor(out=tm[:, sl, :],
                                                in0=tm[:, sl, :],
                                                in1=t2v[:, sl, :], op=ALU.add)
                        # tm = 0.5*tm + bias
                        nc.vector.scalar_tensor_tensor(
                            out=tm[:, sl, :], in0=tm[:, sl, :], scalar=0.5,
                            in1=bias_t[:].unsqueeze(1).to_broadcast(
                                [128, NSb, C]),
                            op0=ALU.mult, op1=ALU.add)
                        # gelu_tanh(x) = x*sigmoid(2*sqrt(2/pi)*(x+0.044715x^3))
                        nc.scalar.square(cv[:, sl, :], tm[:, sl, :])
                        nc.gpsimd.tensor_tensor(out=cv[:, sl, :],
                                                in0=cv[:, sl, :],
                                                in1=tm[:, sl, :], op=ALU.mult)
                        nc.vector.scalar_tensor_tensor(
                            out=cv[:, sl, :], in0=cv[:, sl, :],
                            scalar=0.044715, in1=tm[:, sl, :],
                            op0=ALU.mult, op1=ALU.add)
                        nc.scalar.activation(out=cv[:, sl, :],
                                             in_=cv[:, sl, :],
                                             func=AF.Sigmoid,
                                             scale=1.5957691216057308)
                        nc.vector.tensor_tensor(out=ogv[:, sl, :],
                                                in0=cv[:, sl, :],
                                                in1=tm[:, sl, :], op=ALU.mult)
                        nc.sync.dma_start(
                            out=out_raw[:, p0 * C:p1 * C],
                            in_=tmean[:, p0 * C:p1 * C])
                        nc.sync.dma_start(
                            out=out_act[:, p0 * C:p1 * C],
                            in_=outg[:, p0 * C:p1 * C])

    nc.compile()
    return nc




# ----------------------------------------------------- persistent runner
class Runner:
    """Jit-compiled SPMD callable with reusable device inputs (no donation)."""

    def __init__(self, nc, n_cores):
        import jax
        import concourse.mybir as mb
        from concourse import bass2jax
        from jax.experimental.shard_map import shard_map
        from jax.sharding import Mesh, PartitionSpec
        bass2jax.install_neuronx_cc_hook()
        self.nc = nc
        self.n_cores = n_cores
        in_names, out_names, out_avals, zero_outs = [], [], [], []
        for alloc in nc.m.functions[0].allocations:
            if not isinstance(alloc, mb.MemoryLocationSet):
                continue
            name = alloc.memorylocations[0].name
            if alloc.kind == "ExternalInput":
                in_names.append(name)
            elif alloc.kind == "ExternalOutput":
                out_names.append(name)
                shape = tuple(alloc.tensor_shape)
                dtype = mb.dt.np(alloc.dtype)
                out_avals.append(jax.core.ShapedArray(shape, dtype))
                zero_outs.append(np.zeros(shape, dtype))
        pt = nc.partition_id_tensor
        self.pname = pt.name if pt else None
        if self.pname in in_names:
            in_names.remove(self.pname)
        self.in_names = list(in_names)
        self.out_names = list(out_names)
        self.out_avals = out_avals
        self.zero_outs = zero_outs
        all_in = list(in_names) + list(out_names)
        if self.pname:
            all_in.append(self.pname)

        def _body(*args):
            operands = list(args)
            if self.pname:
                operands.append(bass2jax.partition_id_tensor())
            outs = bass2jax._bass_exec_p.bind(
                *operands,
                out_avals=tuple(out_avals),
                in_names=tuple(all_in),
                out_names=tuple(out_names),
                lowering_input_output_aliases=(),
                sim_require_finite=True,
                sim_require_nnan=True,
                nc=nc,
            )
            return tuple(outs)

        devices = jax.devices()[:n_cores]
        self.mesh = Mesh(np.asarray(devices), ("core",))
        np_in = (PartitionSpec("core"),) * (len(in_names) + len(out_names))
        np_out = (PartitionSpec("core"),) * len(out_names)
        self.fn = jax.jit(shard_map(_body, mesh=self.mesh, in_specs=np_in,
                                    out_specs=np_out, check_rep=False),
                          keep_unused=True)

    def put(self, in_maps):
        """Concat per-core inputs and move to device. Returns arg list."""
        import jax
        from jax.sharding import NamedSharding, PartitionSpec
        sh = NamedSharding(self.mesh, PartitionSpec("core"))
        args = []
        for name in self.in_names:
            cat = np.concatenate([np.asarray(m[name]) for m in in_maps], axis=0)
            args.append(jax.device_put(cat, sh))
        for z in self.zero_outs:
            zz = np.zeros((self.n_cores * z.shape[0], *z.shape[1:]), z.dtype)
            args.append(jax.device_put(zz, sh))
        return args

    def run(self, args):
        return self.fn(*args)

    def results(self, out_arrs):
        res = []
        for c in range(self.n_cores):
            res.append({
                name: np.asarray(out_arrs[i]).reshape(
                    self.n_cores, *self.out_avals[i].shape)[c]
                for i, name in enumerate(self.out_names)})
        return res


# ------------------------------------------------------------- kernel()
_CACHE = {}
_RUNNERS = {}
LAST_ARGS = None
LAST_LAUNCH_NS = None


def build_in_map(cfg, cur, sched, perms, slots, lw, iotaF, dlocP, c):
    NBLK = int(np.sum(sched))
    S = NBLK * 128
    sids, dloc = slots[c]
    pad = sids < 0
    xs = cur[np.where(pad, 0, sids)]
    xs[pad] = 0
    xsT = np.empty((cfg.D + 1, S), np.float16)
    xsT[:cfg.D] = xs.T
    xsT[cfg.D] = (~pad).astype(np.float16)
    # xeP [128, NBLK*66]: partition-major rows of xs
    xeP = np.zeros((NBLK, 128, 66), np.float16)
    xeP[:, :, :cfg.D] = xs.reshape(NBLK, 128, cfg.D)
    xeP[:, :, cfg.D] = (~pad).astype(np.float16).reshape(NBLK, 128)
    xeP = np.ascontiguousarray(
        xeP.transpose(1, 0, 2).reshape(128, NBLK * 66))
    # dst features: sorted strip of each slot -> original strip id
    boff = np.concatenate([[0], np.cumsum(sched)]).astype(np.int64)
    sstrip = np.searchsorted(boff[1:], np.arange(NBLK), side="right")
    strip_of_slot = np.repeat(perms[c][sstrip], 128)
    dlg = np.where(pad, 0, strip_of_slot * cfg.W + dloc)
    xd = cur[c * cfg.RN + np.minimum(dlg, cfg.RN - 1)]
    xd[pad] = 0
    xdT = np.empty((cfg.D + 1, S), np.float16)
    xdT[:cfg.D] = xd.T
    xdT[cfg.D] = (~pad).astype(np.float16)
    return {
        "xsT": xsT, "xdT": xdT, "xeP": xeP,
        "wsl": lw["wsl"], "wsr": lw["wsr"], "A": lw["A"],
        "R2_0": lw["R2_0"], "R2_1": lw["R2_1"],
        "dstloc": dlocP[c], "iotaF": iotaF, "biasF": lw["biasF"],
    }


def prep_all(cfg, src, dst):
    sched, perms, slots = _prep_edges(cfg, src, dst)
    NBLK = int(np.sum(sched))
    dlocP = []
    for c in range(cfg.NCORES):
        sids, dloc = slots[c]
        dlocP.append(np.ascontiguousarray(
            dloc.reshape(NBLK, 128).T.astype(np.float16)))
    return sched, perms, slots, dlocP


def kernel(embeded_nodes_features, edges_connectivity, Wl, bl, Wr, br, att, bias):
    global LAST_LAUNCH_NS
    cfg = CFG
    x = np.asarray(embeded_nodes_features, np.float32)
    ec = np.asarray(edges_connectivity)
    src = np.concatenate([ec[0], np.arange(cfg.N, dtype=ec.dtype)]).astype(np.int64)
    dst = np.concatenate([ec[1], np.arange(cfg.N, dtype=ec.dtype)]).astype(np.int64)
    Wl = np.asarray(Wl, np.float32)
    bl = np.asarray(bl, np.float32)
    Wr = np.asarray(Wr, np.float32)
    br = np.asarray(br, np.float32)
    att = np.asarray(att, np.float32)
    bias = np.asarray(bias, np.float32)
    L = Wl.shape[0]

    sched, perms, slots, dlocP = prep_all(cfg, src, dst)
    key = tuple(int(b) for b in sched)
    if key not in _CACHE:
        _CACHE[key] = build_program(cfg, sched)
    prog = _CACHE[key]
    lws = [_prep_layer_weights(cfg, Wl[i], bl[i], Wr[i], br[i], att[i], bias[i])
           for i in range(L)]
    iotaF = np.tile(np.arange(cfg.W, dtype=np.float16)[None, :], (128, 1))

    cur = x.astype(np.float16)
    out_full = None
    _launch_ns = []
    _args_hist = []
    NS = cfg.NPAIR
    for i in range(L):
        lw = lws[i]
        in_maps = [build_in_map(cfg, cur, sched, perms, slots, lw, iotaF,
                                dlocP, c)
                   for c in range(cfg.NCORES)]
        if key not in _RUNNERS:
            _RUNNERS[key] = Runner(prog, cfg.NCORES)
        runner = _RUNNERS[key]
        args = runner.put(in_maps)
        _args_hist.append(args)
        t0 = time.time()
        outs = runner.run(args)
        import jax
        jax.block_until_ready(outs)
        _launch_ns.append(int((time.time() - t0) * 1e9))
        res = runner.results(outs)

        def unpack(a, c):
            # [128, NS, C] -> sorted strips -> original strip order
            dev = a.reshape(2, cfg.W, NS, cfg.C).transpose(2, 0, 1, 3)
            dev = dev.reshape(cfg.NSTRIP, cfg.W, cfg.C)
            res_ = np.empty_like(dev)
            res_[perms[c]] = dev
            return res_.reshape(cfg.NSTRIP * cfg.W, cfg.C)[:cfg.RN]
        raw = np.concatenate(
            [unpack(res[c]["out_raw"], c) for c in range(cfg.NCORES)], axis=0)
        actv = np.concatenate(
            [unpack(res[c]["out_act"], c) for c in range(cfg.NCORES)], axis=0)
        out_full = raw
        cur = actv
    LAST_LAUNCH_NS = _launch_ns
    global LAST_ARGS
    LAST_ARGS = _args_hist
    return out_full.astype(np.float32)
